# revision 10
# baseline (speedup 1.0000x reference)
"""KAN-LSTM cell Trainium2 kernel (v4: PE k-reduction + t-form activates).

Shapes (hardcoded): B=2048, I=256, H=512, K=32, D=I+H=768, 4 gates.
Sharding: pure data-parallel over batch across 8 cores (B_local=256).

Math per gate g:
  comb = [x | h_prev]                                   [B, D]
  A[b,d,k] = relu(comb[b,d]*W1[d,k] + b1[d,k])
  u[b,d]   = sum_k A*W2[d,k] + b2[d]
  gate     = u @ Wc + bc                                [B, H]
LSTM tail: f,i,o = sigmoid(g0,g1,g2); c~ = tanh(g3)
  c_t = f*c_prev + i*c~ ;  h_t = o*tanh(c_t)

Device formulation (all feature-on-partition, [*, batch] layout):
  The k-replicated comb layout puts 32 features x 4 k-slots on the 128
  partitions: rep[(f,kk), b] = comb[d0+f, b]  (DMA stride-0 broadcast).
  t-form: with t = -b1/W1 and coef = W2*|W1| (W1 clamped away from 0):
    W2*max(W1*c, -b1) = coef*max(c, t)            [W1>0]
                      = coef*max(c, t) + W2*W1*c  [W1<0]
    W2*relu(W1*c+b1)  = coef*relu(c - t)                      [W1>0]
                      = coef*relu(c - t) + W2*W1*c + W2*b1    [W1<0]
  so the activate is ONE single-scalar op per tile:
    DVE path:     M = max(rep, t)           (tensor_scalar, 1 PTR scalar)
    ScalarE path: M = relu(rep + (-t))      (activation, bias PTR, scale=1)
  k-reduce on PE: u[32q:32q+32] += w2l_blk.T @ M (8 k-chunk matmuls, PSUM
  accumulate, tile_position=(0,32q)); the linear residue lambda[d]*c[d]
  (lambda = sum_{k:W1<0} W2*W1) is seeded first as one diagonal matmul
  per (g,t): u = diag(lambda).T @ combT_tile.
  All constant residues (+ sum_k W2*b1 terms + b2) fold into the
  combiner bias on host: bc' = resid @ Wc + bc.
  Combiner: gate[h,b] = sum_t Wc_tile.T @ u_tile (PE, PSUM acc), then
  sigmoid/tanh with bias read PSUM directly on ScalarE.
  Tail in [h, b] layout, bf16; host pre/post-transposes comb, c_prev,
  h_t, c_t (no device transposes at all).
"""

import ml_dtypes
import numpy as np

import concourse.bacc as bacc
import concourse.bass as bass
import concourse.tile as tile
from concourse import mybir
from concourse.bass_utils import run_bass_kernel_spmd

# ---- problem constants ----
B, I, H, K = 2048, 256, 512, 32
D = I + H  # 768
G = 4
NCORES = 8
BL = B // NCORES          # 256 local batch
DT = D // 128             # 6 feature tiles
HT = H // 128             # 4 h tiles
FG = 32                   # features per group
KC = 4                    # k per chunk (FG*KC = 128 partitions)
NKC = K // KC             # 8 k-chunks
NQ = 128 // FG            # 4 groups per dtile
NGRP = D // FG            # 24 rep tiles
NTILE = G * DT * NQ * NKC  # 768 activate tiles
F32 = mybir.dt.float32
BF16 = mybir.dt.bfloat16
BF = ml_dtypes.bfloat16

# tiles whose activate runs on ScalarE (relu path) instead of DVE (max
# path); balance: DVE ~196ns/tile, ScalarE ~440ns/tile -> ~1/3 ScalarE.
ACT_MOD = 3  # idx % ACT_MOD == ACT_MOD-1 -> ScalarE


def _is_act_tile(idx: int) -> bool:
    return ACT_MOD > 0 and idx % ACT_MOD == ACT_MOD - 1

_PROG_CACHE = {}


def _build_program():
    nc = bacc.Bacc(None, target_bir_lowering=False)

    # DRAM I/O (per-core shapes)
    combT = nc.dram_tensor("combT", [D, BL], BF16, kind="ExternalInput")
    ctT = nc.dram_tensor("ctT", [H, BL], BF16, kind="ExternalInput")
    tcol = nc.dram_tensor("tcol", [128, NTILE], F32, kind="ExternalInput")
    w2l = nc.dram_tensor("w2l", [128, NTILE * FG], BF16, kind="ExternalInput")
    lam = nc.dram_tensor("lam", [128, G * DT * 128], BF16,
                         kind="ExternalInput")
    wc = nc.dram_tensor("wc", [128, G * DT * H], BF16, kind="ExternalInput")
    bcp = nc.dram_tensor("bcp", [128, G * HT], F32, kind="ExternalInput")
    hoT = nc.dram_tensor("hoT", [H, BL], BF16, kind="ExternalOutput")
    coT = nc.dram_tensor("coT", [H, BL], BF16, kind="ExternalOutput")

    SIG = mybir.ActivationFunctionType.Sigmoid
    TANH = mybir.ActivationFunctionType.Tanh
    RELU = mybir.ActivationFunctionType.Relu

    with tile.TileContext(nc) as tc:
        with (
            tc.tile_pool(name="const", bufs=1) as const,
            tc.tile_pool(name="repp", bufs=1) as repp,
            tc.tile_pool(name="mp", bufs=12) as mp,
            tc.tile_pool(name="usb", bufs=1) as usb,
            tc.tile_pool(name="gsb", bufs=1) as gsb,
            tc.tile_pool(name="tailp", bufs=2) as tailp,
            tc.tile_pool(name="ups", bufs=3, space="PSUM") as ups,
            tc.tile_pool(name="gps", bufs=1, space="PSUM") as gps,
        ):
            # ---- tiles (DMAs emitted in consumption order below) ----
            tcol_sb = const.tile([128, NTILE], F32, tag="tcol")
            w2l_sb = const.tile([128, NTILE * FG], BF16, tag="w2l")
            lam_sb = const.tile([128, G * DT * 128], BF16, tag="lam")
            wc_sb = const.tile([128, G * DT * H], BF16, tag="wc")
            bcp_sb = const.tile([128, G * HT], F32, tag="bcp")

            # inputs first: combT tiles, rep broadcasts, tcol, c_prev
            cbT = []
            for t in range(DT):
                cb = const.tile([128, BL], BF16, tag=f"cbT{t}",
                                name=f"cbT{t}")
                nc.sync.dma_start(out=cb, in_=combT[t * 128:(t + 1) * 128, :])
                cbT.append(cb)
            # rep[(f,kk), b] = comb[grp*FG + f, b] via stride-0 broadcast
            rep = []
            for grp in range(NGRP):
                r = repp.tile([128, BL], BF16, tag=f"rep{grp}",
                              name=f"rep{grp}")
                d0 = grp * FG
                nc.sync.dma_start(
                    out=r,
                    in_=combT[d0:d0 + FG, None, :].to_broadcast((FG, KC, BL)))
                rep.append(r)
            nc.sync.dma_start(out=tcol_sb, in_=tcol[:, :])
            nc.sync.dma_start(out=bcp_sb, in_=bcp[:, :])
            cT = []
            for j in range(HT):
                c = const.tile([128, BL], BF16, tag=f"cT{j}", name=f"cT{j}")
                nc.sync.dma_start(out=c, in_=ctT[j * 128:(j + 1) * 128, :])
                cT.append(c)

            # per-gate weight slices, in the order compute consumes them
            gw = NTILE * FG // G
            lw = DT * 128
            cw = DT * H
            for g in range(G):
                nc.sync.dma_start(out=w2l_sb[:, g * gw:(g + 1) * gw],
                                  in_=w2l[:, g * gw:(g + 1) * gw])
                nc.sync.dma_start(out=lam_sb[:, g * lw:(g + 1) * lw],
                                  in_=lam[:, g * lw:(g + 1) * lw])
                nc.sync.dma_start(out=wc_sb[:, g * cw:(g + 1) * cw],
                                  in_=wc[:, g * cw:(g + 1) * cw])

            # ---- stage 1 + combiner, per gate ----
            gates_sb = {}
            for g in range(G):
                gps_g = [gps.tile([128, BL], F32, tag=f"gp{hh}",
                                  name=f"gp_{g}_{hh}") for hh in range(HT)]
                for t in range(DT):
                    u_ps = ups.tile([128, BL], F32, tag="u", name=f"u_{g}_{t}")
                    for q in range(NQ):
                        for kc in range(NKC):
                            idx = ((g * DT + t) * NQ + q) * NKC + kc
                            r = rep[t * NQ + q]
                            if _is_act_tile(idx):
                                # M = relu(rep + bias), bias = -t
                                m = mp.tile([128, BL], BF16, tag="ms")
                                nc.scalar.activation(
                                    m, r, RELU,
                                    bias=tcol_sb[:, idx:idx + 1])
                            else:
                                # M = max(rep, t)
                                m = mp.tile([128, BL], BF16, tag="mv")
                                nc.vector.tensor_scalar(
                                    m, r, tcol_sb[:, idx:idx + 1], None,
                                    mybir.AluOpType.max)
                            nc.tensor.matmul(
                                u_ps[FG * q:FG * (q + 1), :],
                                w2l_sb[:, idx * FG:(idx + 1) * FG],
                                m,
                                start=(kc == 0), stop=False,
                                tile_position=(0, FG * q),
                                skip_group_check=True)
                    # linear residue last: u += diag(lambda) @ combT
                    lcol = (g * DT + t) * 128
                    nc.tensor.matmul(u_ps, lam_sb[:, lcol:lcol + 128],
                                     cbT[t], start=False, stop=True,
                                     skip_group_check=True)
                    u_s = usb.tile([128, BL], BF16, tag=f"u_{g}_{t}",
                                   name=f"usb_{g}_{t}")
                    nc.vector.tensor_copy(u_s, u_ps)
                    # combiner contribution of this dtile, all 4 h-tiles
                    for hh in range(HT):
                        lc = g * DT * H + t * H + hh * 128
                        nc.tensor.matmul(gps_g[hh], wc_sb[:, lc:lc + 128],
                                         u_s,
                                         start=(t == 0), stop=(t == DT - 1))

                fn = TANH if g == 3 else SIG
                for hh in range(HT):
                    gs = gsb.tile([128, BL], BF16, tag=f"g_{g}_{hh}",
                                  name=f"gate_{g}_{hh}")
                    col = g * HT + hh
                    nc.scalar.activation(gs, gps_g[hh], fn,
                                         bias=bcp_sb[:, col:col + 1])
                    gates_sb[(g, hh)] = gs

            # ---- LSTM tail in [h, b] layout ----
            for hh in range(HT):
                f = gates_sb[(0, hh)]
                i_ = gates_sb[(1, hh)]
                o = gates_sb[(2, hh)]
                cth = gates_sb[(3, hh)]
                t1 = tailp.tile([128, BL], BF16, tag="t1")
                t2 = tailp.tile([128, BL], BF16, tag="t2")
                ct = tailp.tile([128, BL], BF16, tag="ct", name=f"ct{hh}")
                tch = tailp.tile([128, BL], BF16, tag="tch")
                ht = tailp.tile([128, BL], BF16, tag="ht", name=f"ht{hh}")
                nc.vector.tensor_tensor(t1, f, cT[hh], mybir.AluOpType.mult)
                nc.vector.tensor_tensor(t2, i_, cth, mybir.AluOpType.mult)
                nc.vector.tensor_tensor(ct, t1, t2, mybir.AluOpType.add)
                nc.scalar.activation(tch, ct, TANH)
                nc.vector.tensor_tensor(ht, o, tch, mybir.AluOpType.mult)
                nc.sync.dma_start(out=coT[hh * 128:(hh + 1) * 128, :], in_=ct)
                nc.sync.dma_start(out=hoT[hh * 128:(hh + 1) * 128, :], in_=ht)

    nc.compile()
    return nc


def _host_prep(W1, b1, W2, b2, Wc, bc):
    """Rearrange weights into the t-form tiled layouts."""
    eps = 1e-7
    W1s = np.where(np.abs(W1) < eps, np.where(W1 >= 0, eps, -eps), W1)
    neg = W1s < 0                                     # [G, D, K]
    coef = W2 * np.abs(W1s)                           # lhsT values, all pairs
    t = -b1 / W1s                                     # DVE max-path scalar

    # path mask per (g,d,k): ACT if its tile idx is an ACT tile
    # tile idx = ((g*DT+t)*NQ+q)*NKC+kc ; feature d = t*128+q*FG+f ;
    # k = kc*KC+kk
    gidx, didx, kidx = np.meshgrid(np.arange(G), np.arange(D), np.arange(K),
                                   indexing="ij")
    tt = didx // 128
    qq = (didx % 128) // FG
    kcc = kidx // KC
    tileidx = ((gidx * DT + tt) * NQ + qq) * NKC + kcc
    is_act = (tileidx % ACT_MOD == ACT_MOD - 1) if ACT_MOD > 0 \
        else np.zeros_like(tileidx, dtype=bool)

    # per-tile scalar column: DVE path: t ; ACT path: bias = -t = b1/W1
    scal = np.where(is_act, -t, t)                    # [G, D, K]

    # linear residue lambda[g,d] = sum_{k: W1<0} W2*W1  (both paths)
    lamv = np.where(neg, W2 * W1s, 0.0).sum(-1)       # [G, D]
    # constant residue:
    #  DVE path, W1>0: +W2*b1 ; ACT path, W1<0: +W2*b1 ; else 0
    constv = np.where(~is_act & ~neg, W2 * b1, 0.0).sum(-1) \
        + np.where(is_act & neg, W2 * b1, 0.0).sum(-1) + b2   # [G, D]

    def cols(a):  # [G, D, K] -> [FG*KC, NTILE] per-tile scalar columns
        ar = a.reshape(G, DT, NQ, FG, NKC, KC)
        return np.ascontiguousarray(
            ar.transpose(3, 5, 0, 1, 2, 4).reshape(FG * KC, NTILE))

    tcolh = cols(scal).astype(np.float32)

    # w2l: [128, NTILE*FG], block idx: [p=(f,kk), col f'] = coef if f'==f
    coefc = cols(coef)                                 # [128, NTILE]
    w2blk = np.zeros((FG * KC, NTILE, FG), dtype=np.float32)
    fidx = (np.arange(FG * KC) // KC)
    for p in range(FG * KC):
        w2blk[p, :, fidx[p]] = coefc[p, :]
    w2lh = np.ascontiguousarray(
        w2blk.reshape(FG * KC, NTILE * FG).astype(BF))

    # lam diag blocks: [128, G*DT*128], col (g,t,j) row p:
    #   lamv[g, t*128+p] if p==j else 0
    lamh = np.zeros((128, G * DT * 128), dtype=np.float32)
    lam_r = lamv.reshape(G, DT, 128)
    for g in range(G):
        for t in range(DT):
            base = (g * DT + t) * 128
            np.fill_diagonal(lamh[:, base:base + 128], lam_r[g, t])
    lamh = np.ascontiguousarray(lamh.astype(BF))

    # wc lhsT layout: [128, G*DT*H], col (g,t,h) row p = Wc[g, t*128+p, h]
    wch = np.ascontiguousarray(
        Wc.reshape(G, DT, 128, H).transpose(2, 0, 1, 3)
        .reshape(128, G * DT * H).astype(BF))

    # folded bias: bc' = constv @ Wc + bc
    bcpv = np.einsum('gd,gdh->gh', constv, Wc) + bc   # [G, H]
    bcph = np.ascontiguousarray(
        bcpv.reshape(G, HT, 128).transpose(2, 0, 1).reshape(128, G * HT))
    return tcolh, w2lh, lamh, wch, bcph


def _make_in_maps(x, h_prev, c_prev, W1, b1, W2, b2, Wc, bc):
    x = np.asarray(x, np.float32)
    h_prev = np.asarray(h_prev, np.float32)
    c_prev = np.asarray(c_prev, np.float32)
    tcolh, w2lh, lamh, wch, bcph = _host_prep(
        np.asarray(W1, np.float32), np.asarray(b1, np.float32),
        np.asarray(W2, np.float32), np.asarray(b2, np.float32),
        np.asarray(Wc, np.float32), np.asarray(bc, np.float32))
    combT_all = np.concatenate([x, h_prev], axis=1).T.astype(BF)  # [D, B]
    ctT_all = c_prev.T.astype(BF)                                 # [H, B]

    in_maps = []
    for c in range(NCORES):
        sl = slice(c * BL, (c + 1) * BL)
        in_maps.append({
            "combT": np.ascontiguousarray(combT_all[:, sl]),
            "ctT": np.ascontiguousarray(ctT_all[:, sl]),
            "tcol": tcolh, "w2l": w2lh, "lam": lamh, "wc": wch, "bcp": bcph,
        })
    return in_maps


def kernel(x, h_prev, c_prev, W1, b1, W2, b2, Wc, bc):
    if "prog" not in _PROG_CACHE:
        _PROG_CACHE["prog"] = _build_program()
    nc = _PROG_CACHE["prog"]
    in_maps = _make_in_maps(x, h_prev, c_prev, W1, b1, W2, b2, Wc, bc)
    res = run_bass_kernel_spmd(nc, in_maps, core_ids=list(range(NCORES)))
    h_t = np.concatenate(
        [res.results[c]["hoT"].T.astype(np.float32) for c in range(NCORES)],
        axis=0)
    c_t = np.concatenate(
        [res.results[c]["coT"].T.astype(np.float32) for c in range(NCORES)],
        axis=0)
    return h_t, c_t


# revision 13
# speedup vs baseline: 1.1856x; 1.1856x over previous
"""KAN-LSTM cell Trainium2 kernel (v4: PE k-reduction + t-form activates).

Shapes (hardcoded): B=2048, I=256, H=512, K=32, D=I+H=768, 4 gates.
Sharding: pure data-parallel over batch across 8 cores (B_local=256).

Math per gate g:
  comb = [x | h_prev]                                   [B, D]
  A[b,d,k] = relu(comb[b,d]*W1[d,k] + b1[d,k])
  u[b,d]   = sum_k A*W2[d,k] + b2[d]
  gate     = u @ Wc + bc                                [B, H]
LSTM tail: f,i,o = sigmoid(g0,g1,g2); c~ = tanh(g3)
  c_t = f*c_prev + i*c~ ;  h_t = o*tanh(c_t)

Device formulation (all feature-on-partition, [*, batch] layout):
  The k-replicated comb layout puts 32 features x 4 k-slots on the 128
  partitions: rep[(f,kk), b] = comb[d0+f, b]  (DMA stride-0 broadcast).
  t-form: with t = -b1/W1 and coef = W2*|W1| (W1 clamped away from 0):
    W2*max(W1*c, -b1) = coef*max(c, t)            [W1>0]
                      = coef*max(c, t) + W2*W1*c  [W1<0]
    W2*relu(W1*c+b1)  = coef*relu(c - t)                      [W1>0]
                      = coef*relu(c - t) + W2*W1*c + W2*b1    [W1<0]
  so the activate is ONE single-scalar op per tile:
    DVE path:     M = max(rep, t)           (tensor_scalar, 1 PTR scalar)
    ScalarE path: M = relu(rep + (-t))      (activation, bias PTR, scale=1)
  k-reduce on PE: u[32q:32q+32] += w2l_blk.T @ M (8 k-chunk matmuls, PSUM
  accumulate, tile_position=(0,32q)); the linear residue lambda[d]*c[d]
  (lambda = sum_{k:W1<0} W2*W1) is seeded first as one diagonal matmul
  per (g,t): u = diag(lambda).T @ combT_tile.
  All constant residues (+ sum_k W2*b1 terms + b2) fold into the
  combiner bias on host: bc' = resid @ Wc + bc.
  Combiner: gate[h,b] = sum_t Wc_tile.T @ u_tile (PE, PSUM acc), then
  sigmoid/tanh with bias read PSUM directly on ScalarE.
  Tail in [h, b] layout, bf16; host pre/post-transposes comb, c_prev,
  h_t, c_t (no device transposes at all).
"""

import ml_dtypes
import numpy as np

import concourse.bacc as bacc
import concourse.bass as bass
import concourse.tile as tile
from concourse import mybir
from concourse.bass_utils import run_bass_kernel_spmd

# ---- problem constants ----
B, I, H, K = 2048, 256, 512, 32
D = I + H  # 768
G = 4
NCORES = 8
BL = B // NCORES          # 256 local batch
DT = D // 128             # 6 feature tiles
HT = H // 128             # 4 h tiles
FG = 32                   # features per group
KC = 4                    # k per chunk (FG*KC = 128 partitions)
NKC = K // KC             # 8 k-chunks
NQ = 128 // FG            # 4 groups per dtile
NGRP = D // FG            # 24 rep tiles
NTILE = G * DT * NQ * NKC  # 768 activate tiles
F32 = mybir.dt.float32
BF16 = mybir.dt.bfloat16
BF = ml_dtypes.bfloat16

# tiles whose activate runs on ScalarE (relu path) instead of DVE (max
# path); balance: DVE ~196ns/tile, ScalarE ~440ns/tile -> ~1/3 ScalarE.
ACT_MOD = 3  # idx % ACT_MOD == ACT_MOD-1 -> ScalarE


def _is_act_tile(idx: int) -> bool:
    return ACT_MOD > 0 and idx % ACT_MOD == ACT_MOD - 1

_PROG_CACHE = {}


def _build_program():
    nc = bacc.Bacc(None, target_bir_lowering=False)

    # DRAM I/O (per-core shapes)
    combT = nc.dram_tensor("combT", [D, BL], BF16, kind="ExternalInput")
    repd = nc.dram_tensor("repd", [NGRP * 128, BL], BF16,
                          kind="ExternalInput")
    ctT = nc.dram_tensor("ctT", [H, BL], BF16, kind="ExternalInput")
    tcol = nc.dram_tensor("tcol", [128, NTILE], F32, kind="ExternalInput")
    w2l = nc.dram_tensor("w2l", [128, NTILE * FG], BF16, kind="ExternalInput")
    lam = nc.dram_tensor("lam", [128, G * DT * 128], BF16,
                         kind="ExternalInput")
    wc = nc.dram_tensor("wc", [128, G * DT * H], BF16, kind="ExternalInput")
    bcp = nc.dram_tensor("bcp", [128, G * HT], F32, kind="ExternalInput")
    hoT = nc.dram_tensor("hoT", [H, BL], BF16, kind="ExternalOutput")
    coT = nc.dram_tensor("coT", [H, BL], BF16, kind="ExternalOutput")

    SIG = mybir.ActivationFunctionType.Sigmoid
    TANH = mybir.ActivationFunctionType.Tanh
    RELU = mybir.ActivationFunctionType.Relu

    with tile.TileContext(nc) as tc:
        with (
            tc.tile_pool(name="const", bufs=1) as const,
            tc.tile_pool(name="repp", bufs=1) as repp,
            tc.tile_pool(name="mp", bufs=12) as mp,
            tc.tile_pool(name="usb", bufs=1) as usb,
            tc.tile_pool(name="gsb", bufs=1) as gsb,
            tc.tile_pool(name="tailp", bufs=2) as tailp,
            tc.tile_pool(name="ups", bufs=3, space="PSUM") as ups,
            tc.tile_pool(name="gps", bufs=1, space="PSUM") as gps,
        ):
            # ---- tiles (DMAs emitted in consumption order below) ----
            tcol_sb = const.tile([128, NTILE], F32, tag="tcol")
            w2l_sb = const.tile([128, NTILE * FG], BF16, tag="w2l")
            lam_sb = const.tile([128, G * DT * 128], BF16, tag="lam")
            wc_sb = const.tile([128, G * DT * H], BF16, tag="wc")
            bcp_sb = const.tile([128, G * HT], F32, tag="bcp")

            # inputs first: combT tiles, rep broadcasts, tcol, c_prev
            cbT = []
            for t in range(DT):
                cb = const.tile([128, BL], BF16, tag=f"cbT{t}",
                                name=f"cbT{t}")
                nc.sync.dma_start(out=cb, in_=combT[t * 128:(t + 1) * 128, :])
                cbT.append(cb)
            # rep[(f,kk), b] = comb[grp*FG + f, b], pre-replicated on host
            rep = []
            for grp in range(NGRP):
                r = repp.tile([128, BL], BF16, tag=f"rep{grp}",
                              name=f"rep{grp}")
                nc.sync.dma_start(
                    out=r, in_=repd[grp * 128:(grp + 1) * 128, :])
                rep.append(r)
            nc.sync.dma_start(out=tcol_sb, in_=tcol[:, :])
            nc.sync.dma_start(out=bcp_sb, in_=bcp[:, :])
            cT = []
            for j in range(HT):
                c = const.tile([128, BL], BF16, tag=f"cT{j}", name=f"cT{j}")
                nc.sync.dma_start(out=c, in_=ctT[j * 128:(j + 1) * 128, :])
                cT.append(c)

            # per-gate weight slices, in the order compute consumes them
            gw = NTILE * FG // G
            lw = DT * 128
            cw = DT * H
            for g in range(G):
                nc.sync.dma_start(out=w2l_sb[:, g * gw:(g + 1) * gw],
                                  in_=w2l[:, g * gw:(g + 1) * gw])
                nc.sync.dma_start(out=lam_sb[:, g * lw:(g + 1) * lw],
                                  in_=lam[:, g * lw:(g + 1) * lw])
                nc.sync.dma_start(out=wc_sb[:, g * cw:(g + 1) * cw],
                                  in_=wc[:, g * cw:(g + 1) * cw])

            # ---- stage 1 + combiner, per gate ----
            gates_sb = {}
            for g in range(G):
                gps_g = [gps.tile([128, BL], F32, tag=f"gp{hh}",
                                  name=f"gp_{g}_{hh}") for hh in range(HT)]
                for t in range(DT):
                    u_ps = ups.tile([128, BL], F32, tag="u", name=f"u_{g}_{t}")
                    for q in range(NQ):
                        for kc in range(NKC):
                            idx = ((g * DT + t) * NQ + q) * NKC + kc
                            r = rep[t * NQ + q]
                            if _is_act_tile(idx):
                                # M = relu(rep + bias), bias = -t
                                m = mp.tile([128, BL], BF16, tag="ms")
                                nc.scalar.activation(
                                    m, r, RELU,
                                    bias=tcol_sb[:, idx:idx + 1])
                            else:
                                # M = max(rep, t)
                                m = mp.tile([128, BL], BF16, tag="mv")
                                nc.vector.tensor_scalar(
                                    m, r, tcol_sb[:, idx:idx + 1], None,
                                    mybir.AluOpType.max)
                            nc.tensor.matmul(
                                u_ps[FG * q:FG * (q + 1), :],
                                w2l_sb[:, idx * FG:(idx + 1) * FG],
                                m,
                                start=(kc == 0), stop=False,
                                tile_position=(0, FG * q),
                                skip_group_check=True)
                    # linear residue last: u += diag(lambda) @ combT
                    lcol = (g * DT + t) * 128
                    nc.tensor.matmul(u_ps, lam_sb[:, lcol:lcol + 128],
                                     cbT[t], start=False, stop=True,
                                     skip_group_check=True)
                    u_s = usb.tile([128, BL], BF16, tag=f"u_{g}_{t}",
                                   name=f"usb_{g}_{t}")
                    nc.vector.tensor_copy(u_s, u_ps)
                    # combiner contribution of this dtile, all 4 h-tiles
                    for hh in range(HT):
                        lc = g * DT * H + t * H + hh * 128
                        nc.tensor.matmul(gps_g[hh], wc_sb[:, lc:lc + 128],
                                         u_s,
                                         start=(t == 0), stop=(t == DT - 1))

                fn = TANH if g == 3 else SIG
                for hh in range(HT):
                    gs = gsb.tile([128, BL], BF16, tag=f"g_{g}_{hh}",
                                  name=f"gate_{g}_{hh}")
                    col = g * HT + hh
                    nc.scalar.activation(gs, gps_g[hh], fn,
                                         bias=bcp_sb[:, col:col + 1])
                    gates_sb[(g, hh)] = gs

            # ---- LSTM tail in [h, b] layout ----
            for hh in range(HT):
                f = gates_sb[(0, hh)]
                i_ = gates_sb[(1, hh)]
                o = gates_sb[(2, hh)]
                cth = gates_sb[(3, hh)]
                t1 = tailp.tile([128, BL], BF16, tag="t1")
                t2 = tailp.tile([128, BL], BF16, tag="t2")
                ct = tailp.tile([128, BL], BF16, tag="ct", name=f"ct{hh}")
                tch = tailp.tile([128, BL], BF16, tag="tch")
                ht = tailp.tile([128, BL], BF16, tag="ht", name=f"ht{hh}")
                nc.vector.tensor_tensor(t1, f, cT[hh], mybir.AluOpType.mult)
                nc.vector.tensor_tensor(t2, i_, cth, mybir.AluOpType.mult)
                nc.vector.tensor_tensor(ct, t1, t2, mybir.AluOpType.add)
                nc.scalar.activation(tch, ct, TANH)
                nc.vector.tensor_tensor(ht, o, tch, mybir.AluOpType.mult)
                nc.sync.dma_start(out=coT[hh * 128:(hh + 1) * 128, :], in_=ct)
                nc.sync.dma_start(out=hoT[hh * 128:(hh + 1) * 128, :], in_=ht)

    nc.compile()
    return nc


def _host_prep(W1, b1, W2, b2, Wc, bc):
    """Rearrange weights into the t-form tiled layouts."""
    eps = 1e-7
    W1s = np.where(np.abs(W1) < eps, np.where(W1 >= 0, eps, -eps), W1)
    neg = W1s < 0                                     # [G, D, K]
    coef = W2 * np.abs(W1s)                           # lhsT values, all pairs
    t = -b1 / W1s                                     # DVE max-path scalar

    # path mask per (g,d,k): ACT if its tile idx is an ACT tile
    # tile idx = ((g*DT+t)*NQ+q)*NKC+kc ; feature d = t*128+q*FG+f ;
    # k = kc*KC+kk
    gidx, didx, kidx = np.meshgrid(np.arange(G), np.arange(D), np.arange(K),
                                   indexing="ij")
    tt = didx // 128
    qq = (didx % 128) // FG
    kcc = kidx // KC
    tileidx = ((gidx * DT + tt) * NQ + qq) * NKC + kcc
    is_act = (tileidx % ACT_MOD == ACT_MOD - 1) if ACT_MOD > 0 \
        else np.zeros_like(tileidx, dtype=bool)

    # per-tile scalar column: DVE path: t ; ACT path: bias = -t = b1/W1
    scal = np.where(is_act, -t, t)                    # [G, D, K]

    # linear residue lambda[g,d] = sum_{k: W1<0} W2*W1  (both paths)
    lamv = np.where(neg, W2 * W1s, 0.0).sum(-1)       # [G, D]
    # constant residue:
    #  DVE path, W1>0: +W2*b1 ; ACT path, W1<0: +W2*b1 ; else 0
    constv = np.where(~is_act & ~neg, W2 * b1, 0.0).sum(-1) \
        + np.where(is_act & neg, W2 * b1, 0.0).sum(-1) + b2   # [G, D]

    def cols(a):  # [G, D, K] -> [FG*KC, NTILE] per-tile scalar columns
        ar = a.reshape(G, DT, NQ, FG, NKC, KC)
        return np.ascontiguousarray(
            ar.transpose(3, 5, 0, 1, 2, 4).reshape(FG * KC, NTILE))

    tcolh = cols(scal).astype(np.float32)

    # w2l: [128, NTILE*FG], block idx: [p=(f,kk), col f'] = coef if f'==f
    coefc = cols(coef)                                 # [128, NTILE]
    w2blk = np.zeros((FG * KC, NTILE, FG), dtype=np.float32)
    fidx = (np.arange(FG * KC) // KC)
    for p in range(FG * KC):
        w2blk[p, :, fidx[p]] = coefc[p, :]
    w2lh = np.ascontiguousarray(
        w2blk.reshape(FG * KC, NTILE * FG).astype(BF))

    # lam diag blocks: [128, G*DT*128], col (g,t,j) row p:
    #   lamv[g, t*128+p] if p==j else 0
    lamh = np.zeros((128, G * DT * 128), dtype=np.float32)
    lam_r = lamv.reshape(G, DT, 128)
    for g in range(G):
        for t in range(DT):
            base = (g * DT + t) * 128
            np.fill_diagonal(lamh[:, base:base + 128], lam_r[g, t])
    lamh = np.ascontiguousarray(lamh.astype(BF))

    # wc lhsT layout: [128, G*DT*H], col (g,t,h) row p = Wc[g, t*128+p, h]
    wch = np.ascontiguousarray(
        Wc.reshape(G, DT, 128, H).transpose(2, 0, 1, 3)
        .reshape(128, G * DT * H).astype(BF))

    # folded bias: bc' = constv @ Wc + bc
    bcpv = np.einsum('gd,gdh->gh', constv, Wc) + bc   # [G, H]
    bcph = np.ascontiguousarray(
        bcpv.reshape(G, HT, 128).transpose(2, 0, 1).reshape(128, G * HT))
    return tcolh, w2lh, lamh, wch, bcph


def _make_in_maps(x, h_prev, c_prev, W1, b1, W2, b2, Wc, bc):
    x = np.asarray(x, np.float32)
    h_prev = np.asarray(h_prev, np.float32)
    c_prev = np.asarray(c_prev, np.float32)
    tcolh, w2lh, lamh, wch, bcph = _host_prep(
        np.asarray(W1, np.float32), np.asarray(b1, np.float32),
        np.asarray(W2, np.float32), np.asarray(b2, np.float32),
        np.asarray(Wc, np.float32), np.asarray(bc, np.float32))
    combT_all = np.concatenate([x, h_prev], axis=1).T.astype(BF)  # [D, B]
    ctT_all = c_prev.T.astype(BF)                                 # [H, B]

    in_maps = []
    for c in range(NCORES):
        sl = slice(c * BL, (c + 1) * BL)
        combT_c = np.ascontiguousarray(combT_all[:, sl])
        in_maps.append({
            "combT": combT_c,
            "repd": np.ascontiguousarray(np.repeat(combT_c, KC, axis=0)),
            "ctT": np.ascontiguousarray(ctT_all[:, sl]),
            "tcol": tcolh, "w2l": w2lh, "lam": lamh, "wc": wch, "bcp": bcph,
        })
    return in_maps


def kernel(x, h_prev, c_prev, W1, b1, W2, b2, Wc, bc):
    if "prog" not in _PROG_CACHE:
        _PROG_CACHE["prog"] = _build_program()
    nc = _PROG_CACHE["prog"]
    in_maps = _make_in_maps(x, h_prev, c_prev, W1, b1, W2, b2, Wc, bc)
    res = run_bass_kernel_spmd(nc, in_maps, core_ids=list(range(NCORES)))
    h_t = np.concatenate(
        [res.results[c]["hoT"].T.astype(np.float32) for c in range(NCORES)],
        axis=0)
    c_t = np.concatenate(
        [res.results[c]["coT"].T.astype(np.float32) for c in range(NCORES)],
        axis=0)
    return h_t, c_t


# revision 14
# speedup vs baseline: 1.2139x; 1.0239x over previous
"""KAN-LSTM cell Trainium2 kernel (v4: PE k-reduction + t-form activates).

Shapes (hardcoded): B=2048, I=256, H=512, K=32, D=I+H=768, 4 gates.
Sharding: pure data-parallel over batch across 8 cores (B_local=256).

Math per gate g:
  comb = [x | h_prev]                                   [B, D]
  A[b,d,k] = relu(comb[b,d]*W1[d,k] + b1[d,k])
  u[b,d]   = sum_k A*W2[d,k] + b2[d]
  gate     = u @ Wc + bc                                [B, H]
LSTM tail: f,i,o = sigmoid(g0,g1,g2); c~ = tanh(g3)
  c_t = f*c_prev + i*c~ ;  h_t = o*tanh(c_t)

Device formulation (all feature-on-partition, [*, batch] layout):
  The k-replicated comb layout puts 32 features x 4 k-slots on the 128
  partitions: rep[(f,kk), b] = comb[d0+f, b]  (DMA stride-0 broadcast).
  t-form: with t = -b1/W1 and coef = W2*|W1| (W1 clamped away from 0):
    W2*max(W1*c, -b1) = coef*max(c, t)            [W1>0]
                      = coef*max(c, t) + W2*W1*c  [W1<0]
    W2*relu(W1*c+b1)  = coef*relu(c - t)                      [W1>0]
                      = coef*relu(c - t) + W2*W1*c + W2*b1    [W1<0]
  so the activate is ONE single-scalar op per tile:
    DVE path:     M = max(rep, t)           (tensor_scalar, 1 PTR scalar)
    ScalarE path: M = relu(rep + (-t))      (activation, bias PTR, scale=1)
  k-reduce on PE: u[32q:32q+32] += w2l_blk.T @ M (8 k-chunk matmuls, PSUM
  accumulate, tile_position=(0,32q)); the linear residue lambda[d]*c[d]
  (lambda = sum_{k:W1<0} W2*W1) is seeded first as one diagonal matmul
  per (g,t): u = diag(lambda).T @ combT_tile.
  All constant residues (+ sum_k W2*b1 terms + b2) fold into the
  combiner bias on host: bc' = resid @ Wc + bc.
  Combiner: gate[h,b] = sum_t Wc_tile.T @ u_tile (PE, PSUM acc), then
  sigmoid/tanh with bias read PSUM directly on ScalarE.
  Tail in [h, b] layout, bf16; host pre/post-transposes comb, c_prev,
  h_t, c_t (no device transposes at all).
"""

import ml_dtypes
import numpy as np

import concourse.bacc as bacc
import concourse.bass as bass
import concourse.tile as tile
from concourse import mybir
from concourse.bass_utils import run_bass_kernel_spmd

# ---- problem constants ----
B, I, H, K = 2048, 256, 512, 32
D = I + H  # 768
G = 4
NCORES = 8
BL = B // NCORES          # 256 local batch
DT = D // 128             # 6 feature tiles
HT = H // 128             # 4 h tiles
FG = 32                   # features per group
KC = 4                    # k per chunk (FG*KC = 128 partitions)
NKC = K // KC             # 8 k-chunks
NQ = 128 // FG            # 4 groups per dtile
NGRP = D // FG            # 24 rep tiles
NTILE = G * DT * NQ * NKC  # 768 activate tiles
F32 = mybir.dt.float32
BF16 = mybir.dt.bfloat16
BF = ml_dtypes.bfloat16

# tiles whose activate runs on ScalarE (relu path) instead of DVE (max
# path); balance: DVE ~196ns/tile, ScalarE ~440ns/tile -> ~1/3 ScalarE.
ACT_MOD = 3  # idx % ACT_MOD == ACT_MOD-1 -> ScalarE


def _is_act_tile(idx: int) -> bool:
    return ACT_MOD > 0 and idx % ACT_MOD == ACT_MOD - 1

_PROG_CACHE = {}


def _build_program():
    nc = bacc.Bacc(None, target_bir_lowering=False)

    # DRAM I/O (per-core shapes)
    combT = nc.dram_tensor("combT", [D, BL], BF16, kind="ExternalInput")
    repd = nc.dram_tensor("repd", [NGRP * 128, BL], BF16,
                          kind="ExternalInput")
    ctT = nc.dram_tensor("ctT", [H, BL], BF16, kind="ExternalInput")
    tcol = nc.dram_tensor("tcol", [128, NTILE], F32, kind="ExternalInput")
    w2l = nc.dram_tensor("w2l", [128, NTILE * FG], BF16, kind="ExternalInput")
    lam = nc.dram_tensor("lam", [128, G * DT * 128], BF16,
                         kind="ExternalInput")
    wc = nc.dram_tensor("wc", [128, G * DT * H], BF16, kind="ExternalInput")
    bcp = nc.dram_tensor("bcp", [128, G * HT], F32, kind="ExternalInput")
    hoT = nc.dram_tensor("hoT", [H, BL], BF16, kind="ExternalOutput")
    coT = nc.dram_tensor("coT", [H, BL], BF16, kind="ExternalOutput")

    SIG = mybir.ActivationFunctionType.Sigmoid
    TANH = mybir.ActivationFunctionType.Tanh
    RELU = mybir.ActivationFunctionType.Relu

    with tile.TileContext(nc) as tc:
        with (
            tc.tile_pool(name="const", bufs=1) as const,
            tc.tile_pool(name="repp", bufs=1) as repp,
            tc.tile_pool(name="mp", bufs=12) as mp,
            tc.tile_pool(name="usb", bufs=1) as usb,
            tc.tile_pool(name="gsb", bufs=1) as gsb,
            tc.tile_pool(name="tailp", bufs=2) as tailp,
            tc.tile_pool(name="ups", bufs=3, space="PSUM") as ups,
            tc.tile_pool(name="gps", bufs=1, space="PSUM") as gps,
        ):
            # ---- tiles (DMAs emitted in consumption order below) ----
            tcol_sb = const.tile([128, NTILE], F32, tag="tcol")
            w2l_sb = const.tile([128, NTILE * FG], BF16, tag="w2l")
            lam_sb = const.tile([128, G * DT * 128], BF16, tag="lam")
            wc_sb = const.tile([128, G * DT * H], BF16, tag="wc")
            bcp_sb = const.tile([128, G * HT], F32, tag="bcp")

            # inputs on the Sync queue: tcol first (gates the first
            # activate), then rep / combT / c_prev as single big DMAs
            nc.sync.dma_start(out=tcol_sb, in_=tcol[:, :])
            rep_big = repp.tile([128, NGRP, BL], BF16, tag="rep",
                                name="rep_big")
            nc.sync.dma_start(out=rep_big,
                              in_=repd.rearrange("(g p) b -> p g b", p=128))
            rep = [rep_big[:, grp, :] for grp in range(NGRP)]
            cb_big = const.tile([128, DT, BL], BF16, tag="cbT",
                                name="cb_big")
            nc.sync.dma_start(out=cb_big,
                              in_=combT.rearrange("(t p) b -> p t b", p=128))
            cbT = [cb_big[:, t, :] for t in range(DT)]
            nc.sync.dma_start(out=bcp_sb, in_=bcp[:, :])
            c_big = const.tile([128, HT, BL], BF16, tag="cT", name="c_big")
            nc.sync.dma_start(out=c_big,
                              in_=ctT.rearrange("(j p) b -> p j b", p=128))
            cT = [c_big[:, j, :] for j in range(HT)]

            # per-gate weight slices on the Scalar HWDGE queue (parallel
            # issue), in the order compute consumes them
            gw = NTILE * FG // G
            lw = DT * 128
            cw = DT * H
            for g in range(G):
                nc.scalar.dma_start(out=w2l_sb[:, g * gw:(g + 1) * gw],
                                    in_=w2l[:, g * gw:(g + 1) * gw])
                nc.scalar.dma_start(out=lam_sb[:, g * lw:(g + 1) * lw],
                                    in_=lam[:, g * lw:(g + 1) * lw])
                nc.scalar.dma_start(out=wc_sb[:, g * cw:(g + 1) * cw],
                                    in_=wc[:, g * cw:(g + 1) * cw])

            # ---- stage 1 + combiner, per gate ----
            gates_sb = {}
            for g in range(G):
                gps_g = [gps.tile([128, BL], F32, tag=f"gp{hh}",
                                  name=f"gp_{g}_{hh}") for hh in range(HT)]
                for t in range(DT):
                    u_ps = ups.tile([128, BL], F32, tag="u", name=f"u_{g}_{t}")
                    for q in range(NQ):
                        for kc in range(NKC):
                            idx = ((g * DT + t) * NQ + q) * NKC + kc
                            r = rep[t * NQ + q]
                            if _is_act_tile(idx):
                                # M = relu(rep + bias), bias = -t
                                m = mp.tile([128, BL], BF16, tag="ms")
                                nc.scalar.activation(
                                    m, r, RELU,
                                    bias=tcol_sb[:, idx:idx + 1])
                            else:
                                # M = max(rep, t)
                                m = mp.tile([128, BL], BF16, tag="mv")
                                nc.vector.tensor_scalar(
                                    m, r, tcol_sb[:, idx:idx + 1], None,
                                    mybir.AluOpType.max)
                            nc.tensor.matmul(
                                u_ps[FG * q:FG * (q + 1), :],
                                w2l_sb[:, idx * FG:(idx + 1) * FG],
                                m,
                                start=(kc == 0), stop=False,
                                tile_position=(0, FG * q),
                                skip_group_check=True)
                    # linear residue last: u += diag(lambda) @ combT
                    lcol = (g * DT + t) * 128
                    nc.tensor.matmul(u_ps, lam_sb[:, lcol:lcol + 128],
                                     cbT[t], start=False, stop=True,
                                     skip_group_check=True)
                    u_s = usb.tile([128, BL], BF16, tag=f"u_{g}_{t}",
                                   name=f"usb_{g}_{t}")
                    nc.vector.tensor_copy(u_s, u_ps)
                    # combiner contribution of this dtile, all 4 h-tiles
                    for hh in range(HT):
                        lc = g * DT * H + t * H + hh * 128
                        nc.tensor.matmul(gps_g[hh], wc_sb[:, lc:lc + 128],
                                         u_s,
                                         start=(t == 0), stop=(t == DT - 1))

                fn = TANH if g == 3 else SIG
                for hh in range(HT):
                    gs = gsb.tile([128, BL], BF16, tag=f"g_{g}_{hh}",
                                  name=f"gate_{g}_{hh}")
                    col = g * HT + hh
                    nc.scalar.activation(gs, gps_g[hh], fn,
                                         bias=bcp_sb[:, col:col + 1])
                    gates_sb[(g, hh)] = gs

            # ---- LSTM tail in [h, b] layout ----
            for hh in range(HT):
                f = gates_sb[(0, hh)]
                i_ = gates_sb[(1, hh)]
                o = gates_sb[(2, hh)]
                cth = gates_sb[(3, hh)]
                t1 = tailp.tile([128, BL], BF16, tag="t1")
                t2 = tailp.tile([128, BL], BF16, tag="t2")
                ct = tailp.tile([128, BL], BF16, tag="ct", name=f"ct{hh}")
                tch = tailp.tile([128, BL], BF16, tag="tch")
                ht = tailp.tile([128, BL], BF16, tag="ht", name=f"ht{hh}")
                nc.vector.tensor_tensor(t1, f, cT[hh], mybir.AluOpType.mult)
                nc.vector.tensor_tensor(t2, i_, cth, mybir.AluOpType.mult)
                nc.vector.tensor_tensor(ct, t1, t2, mybir.AluOpType.add)
                nc.scalar.activation(tch, ct, TANH)
                nc.vector.tensor_tensor(ht, o, tch, mybir.AluOpType.mult)
                nc.sync.dma_start(out=coT[hh * 128:(hh + 1) * 128, :], in_=ct)
                nc.sync.dma_start(out=hoT[hh * 128:(hh + 1) * 128, :], in_=ht)

    nc.compile()
    return nc


def _host_prep(W1, b1, W2, b2, Wc, bc):
    """Rearrange weights into the t-form tiled layouts."""
    eps = 1e-7
    W1s = np.where(np.abs(W1) < eps, np.where(W1 >= 0, eps, -eps), W1)
    neg = W1s < 0                                     # [G, D, K]
    coef = W2 * np.abs(W1s)                           # lhsT values, all pairs
    t = -b1 / W1s                                     # DVE max-path scalar

    # path mask per (g,d,k): ACT if its tile idx is an ACT tile
    # tile idx = ((g*DT+t)*NQ+q)*NKC+kc ; feature d = t*128+q*FG+f ;
    # k = kc*KC+kk
    gidx, didx, kidx = np.meshgrid(np.arange(G), np.arange(D), np.arange(K),
                                   indexing="ij")
    tt = didx // 128
    qq = (didx % 128) // FG
    kcc = kidx // KC
    tileidx = ((gidx * DT + tt) * NQ + qq) * NKC + kcc
    is_act = (tileidx % ACT_MOD == ACT_MOD - 1) if ACT_MOD > 0 \
        else np.zeros_like(tileidx, dtype=bool)

    # per-tile scalar column: DVE path: t ; ACT path: bias = -t = b1/W1
    scal = np.where(is_act, -t, t)                    # [G, D, K]

    # linear residue lambda[g,d] = sum_{k: W1<0} W2*W1  (both paths)
    lamv = np.where(neg, W2 * W1s, 0.0).sum(-1)       # [G, D]
    # constant residue:
    #  DVE path, W1>0: +W2*b1 ; ACT path, W1<0: +W2*b1 ; else 0
    constv = np.where(~is_act & ~neg, W2 * b1, 0.0).sum(-1) \
        + np.where(is_act & neg, W2 * b1, 0.0).sum(-1) + b2   # [G, D]

    def cols(a):  # [G, D, K] -> [FG*KC, NTILE] per-tile scalar columns
        ar = a.reshape(G, DT, NQ, FG, NKC, KC)
        return np.ascontiguousarray(
            ar.transpose(3, 5, 0, 1, 2, 4).reshape(FG * KC, NTILE))

    tcolh = cols(scal).astype(np.float32)

    # w2l: [128, NTILE*FG], block idx: [p=(f,kk), col f'] = coef if f'==f
    coefc = cols(coef)                                 # [128, NTILE]
    w2blk = np.zeros((FG * KC, NTILE, FG), dtype=np.float32)
    fidx = (np.arange(FG * KC) // KC)
    for p in range(FG * KC):
        w2blk[p, :, fidx[p]] = coefc[p, :]
    w2lh = np.ascontiguousarray(
        w2blk.reshape(FG * KC, NTILE * FG).astype(BF))

    # lam diag blocks: [128, G*DT*128], col (g,t,j) row p:
    #   lamv[g, t*128+p] if p==j else 0
    lamh = np.zeros((128, G * DT * 128), dtype=np.float32)
    lam_r = lamv.reshape(G, DT, 128)
    for g in range(G):
        for t in range(DT):
            base = (g * DT + t) * 128
            np.fill_diagonal(lamh[:, base:base + 128], lam_r[g, t])
    lamh = np.ascontiguousarray(lamh.astype(BF))

    # wc lhsT layout: [128, G*DT*H], col (g,t,h) row p = Wc[g, t*128+p, h]
    wch = np.ascontiguousarray(
        Wc.reshape(G, DT, 128, H).transpose(2, 0, 1, 3)
        .reshape(128, G * DT * H).astype(BF))

    # folded bias: bc' = constv @ Wc + bc
    bcpv = np.einsum('gd,gdh->gh', constv, Wc) + bc   # [G, H]
    bcph = np.ascontiguousarray(
        bcpv.reshape(G, HT, 128).transpose(2, 0, 1).reshape(128, G * HT))
    return tcolh, w2lh, lamh, wch, bcph


def _make_in_maps(x, h_prev, c_prev, W1, b1, W2, b2, Wc, bc):
    x = np.asarray(x, np.float32)
    h_prev = np.asarray(h_prev, np.float32)
    c_prev = np.asarray(c_prev, np.float32)
    tcolh, w2lh, lamh, wch, bcph = _host_prep(
        np.asarray(W1, np.float32), np.asarray(b1, np.float32),
        np.asarray(W2, np.float32), np.asarray(b2, np.float32),
        np.asarray(Wc, np.float32), np.asarray(bc, np.float32))
    combT_all = np.concatenate([x, h_prev], axis=1).T.astype(BF)  # [D, B]
    ctT_all = c_prev.T.astype(BF)                                 # [H, B]

    in_maps = []
    for c in range(NCORES):
        sl = slice(c * BL, (c + 1) * BL)
        combT_c = np.ascontiguousarray(combT_all[:, sl])
        in_maps.append({
            "combT": combT_c,
            "repd": np.ascontiguousarray(np.repeat(combT_c, KC, axis=0)),
            "ctT": np.ascontiguousarray(ctT_all[:, sl]),
            "tcol": tcolh, "w2l": w2lh, "lam": lamh, "wc": wch, "bcp": bcph,
        })
    return in_maps


def kernel(x, h_prev, c_prev, W1, b1, W2, b2, Wc, bc):
    if "prog" not in _PROG_CACHE:
        _PROG_CACHE["prog"] = _build_program()
    nc = _PROG_CACHE["prog"]
    in_maps = _make_in_maps(x, h_prev, c_prev, W1, b1, W2, b2, Wc, bc)
    res = run_bass_kernel_spmd(nc, in_maps, core_ids=list(range(NCORES)))
    h_t = np.concatenate(
        [res.results[c]["hoT"].T.astype(np.float32) for c in range(NCORES)],
        axis=0)
    c_t = np.concatenate(
        [res.results[c]["coT"].T.astype(np.float32) for c in range(NCORES)],
        axis=0)
    return h_t, c_t
